# revision 11
# baseline (speedup 1.0000x reference)
"""Cross-attention block (B=16, N=4096 queries, M=77 keys, 8 heads x 64) on 8 trn2 cores.

Sharding: data-parallel over batch; each core gets 2 batches, full weights.

v2 dataflow per core (matmuls bf16 in / fp32 psum):
  x -> per-chunk bf16 staging copies in DRAM (gpsimd cast DMA, issued up front)
  xbar-transpose-loaded as xT [feat, tok] (sync HWDGE).
  qT = Wq.T @ xT                    (weight-stationary, 2-bank psum groups)
  per head pair: row-tiled score matmuls (heads at PE partitions 0:64/64:128)
  -> exp via one [77,1024] ScalarE activation -> E
  denominators: 8 picker-matmuls accumulate colsum(E_h) into psum [8, tok],
  reciprocal_approx_fast -> bf16, then PE broadcast matmuls (selector
  stationary) replicate 1/sum across partitions -- no DRAM bounce.
  aT = (v_h.T @ E_h) * recip  (DVE mul reads both operands straight from PSUM)
  out = aT.T @ Wo + bo              (aT stationary -> token-major psum)
"""

import numpy as np

import concourse.bass as bass
import concourse.mybir as mybir
import concourse.tile as tile
from concourse import bacc
from concourse._compat import with_exitstack
from concourse.bass_utils import run_bass_kernel_spmd
from concourse.masks import make_identity
from contextlib import ExitStack

N_CORES = 8
B, N, FEAT, CD = 16, 4096, 512, 768
M = 77          # cond tokens
H, DH = 8, 64
DA = H * DH     # 512
BP = B // N_CORES   # batches per core
TC = 512            # token chunk
NT = N // TC        # chunks per batch
SUB = TC // 128     # 128-token subtiles per chunk
KC = FEAT // 128    # x feature chunks
CC = CD // 128      # cond feature chunks
MC = DA // 128      # d_attn chunks
HPAIRS = H // 2

F32 = mybir.dt.float32
BF16 = mybir.dt.bfloat16
FP8 = mybir.dt.float8e4
EXP = mybir.ActivationFunctionType.Exp
DR = mybir.MatmulPerfMode.DoubleRow

import os
FP8_Q = os.environ.get("K_FP8_Q", "1") == "1"   # fp8 DoubleRow q-projection
FP8_O = os.environ.get("K_FP8_O", "1") == "1"   # fp8 DoubleRow out-projection


@with_exitstack
def _body(ctx: ExitStack, tc: tile.TileContext, x, x_bf, cond, Wq, Wk, Wv, Wo, bo, out):
    nc = tc.nc

    wpool = ctx.enter_context(tc.tile_pool(name="wpool", bufs=1))
    Wq_bf = wpool.tile([128, KC, DA], BF16, tag="wq")
    Wk_bf = wpool.tile([128, CC, DA], BF16, tag="wk")
    Wv_bf = wpool.tile([128, CC, DA], BF16, tag="wv")
    Wo_bf = wpool.tile([128, MC, FEAT], BF16, tag="wo")
    bo_bc = wpool.tile([128, FEAT], F32, tag="bo")
    ident = wpool.tile([128, 128], F32, tag="ident")
    # 0/1 picker: col 8 is ones; colpick[:, 8-h : 16-h] selects head h
    colpick = wpool.tile([128, 17], BF16, tag="colpick")
    # per-pair broadcast selectors: sel[k, hp, i] = (k == 2*hp + (i >= 64))
    sel = wpool.tile([8, HPAIRS, 128], BF16, tag="sel")

    for k in range(KC):
        nc.gpsimd.dma_start(out=Wq_bf[:, k, :], in_=Wq[128 * k : 128 * (k + 1), :])
    for c in range(CC):
        nc.gpsimd.dma_start(out=Wk_bf[:, c, :], in_=Wk[128 * c : 128 * (c + 1), :])
        nc.gpsimd.dma_start(out=Wv_bf[:, c, :], in_=Wv[128 * c : 128 * (c + 1), :])
    for m in range(MC):
        nc.gpsimd.dma_start(out=Wo_bf[:, m, :], in_=Wo[128 * m : 128 * (m + 1), :])
    bo_bcast_ap = bass.AP(tensor=bo.tensor, offset=bo.offset, ap=[[0, 128], *bo.ap])
    nc.gpsimd.dma_start(out=bo_bc[:, :], in_=bo_bcast_ap)
    make_identity(nc, ident)
    nc.gpsimd.memset(colpick[:, :], 0.0)
    nc.gpsimd.memset(colpick[:, 8:9], 1.0)
    # sel[k, hp, i] = (k == 2*hp + (i >= 64)): ones everywhere, then keep the
    # diagonal k==2hp on the low half / k==2hp+1 on the high half
    nc.gpsimd.memset(sel[:, :, :], 1.0)
    nc.gpsimd.affine_select(
        out=sel[:, :, 0:64],
        in_=sel[:, :, 0:64],
        pattern=[[-2, HPAIRS], [0, 64]],
        channel_multiplier=1,
        base=0,
        compare_op=mybir.AluOpType.is_equal,
        fill=0.0,
    )
    nc.gpsimd.affine_select(
        out=sel[:, :, 64:128],
        in_=sel[:, :, 64:128],
        pattern=[[-2, HPAIRS], [0, 64]],
        channel_multiplier=1,
        base=-1,
        compare_op=mybir.AluOpType.is_equal,
        fill=0.0,
    )

    # per-chunk bf16 staging copies of x (transpose-loads need a 2-byte dtype)
    for b in range(BP):
        for t in range(NT):
            nc.gpsimd.dma_start(
                out=x_bf[b, TC * t : TC * (t + 1), :],
                in_=x[b, TC * t : TC * (t + 1), :],
            )

    bpool = ctx.enter_context(tc.tile_pool(name="bpool", bufs=2))
    xpool = ctx.enter_context(tc.tile_pool(name="xpool", bufs=3))
    qpool = ctx.enter_context(tc.tile_pool(name="qpool", bufs=2))
    epool = ctx.enter_context(tc.tile_pool(name="epool", bufs=2))
    rpool = ctx.enter_context(tc.tile_pool(name="rpool", bufs=2))
    apool = ctx.enter_context(tc.tile_pool(name="apool", bufs=2))
    opool = ctx.enter_context(tc.tile_pool(name="opool", bufs=4))

    # psum bank budget (8): mm 2x2 + pq(shared with sm) 1x2 + pb 1x2 = 8
    pmain = ctx.enter_context(tc.tile_pool(name="pmain", bufs=2, space="PSUM"))
    pqp = ctx.enter_context(tc.tile_pool(name="pqp", bufs=1, space="PSUM"))
    psbp = ctx.enter_context(tc.tile_pool(name="psbp", bufs=1, space="PSUM"))

    for b in range(BP):
        # cond[b] -> cond.T (PE transpose) -> K/V projections
        cond_sb = bpool.tile([128, CD], F32, tag="cond")
        nc.sync.dma_start(out=cond_sb[:M, :], in_=cond[b, :, :])
        condT = bpool.tile([128, CC, M], BF16, tag="condT")
        for c in range(CC):
            ps = psbp.tile([128, TC], F32, tag="pb")
            nc.tensor.matmul(
                ps[:128, :M],
                cond_sb[:M, 128 * c : 128 * (c + 1)],
                ident[:M, :M],
                is_transpose=True,
            )
            nc.scalar.copy(condT[:, c, :], ps[:128, :M])

        # kT[d_attn, M] = Wk.T @ cond.T
        kT = bpool.tile([128, MC, M], BF16, tag="kT")
        for m in range(MC):
            pk = psbp.tile([128, TC], F32, tag="pb")
            for c in range(CC):
                nc.tensor.matmul(
                    pk[:, :M],
                    Wk_bf[:, c, 128 * m : 128 * (m + 1)],
                    condT[:, c, :],
                    start=(c == 0),
                    stop=(c == CC - 1),
                )
            nc.scalar.copy(kT[:, m, :], pk[:, :M])

        # v[M, d_attn] = cond @ Wv  (cond.T is the stationary operand)
        pv = psbp.tile([128, TC], F32, tag="pb")
        for c in range(CC):
            nc.tensor.matmul(
                pv[:M, :],
                condT[:, c, :],
                Wv_bf[:, c, :],
                start=(c == 0),
                stop=(c == CC - 1),
            )
        v_bf = bpool.tile([128, DA], BF16, tag="v")
        nc.scalar.copy(v_bf[:M, :], pv[:M, :])

        for t in range(NT):
            tok0 = t * TC
            # xT[feat, tok] via one fused xbar transpose of the whole chunk
            # (chunk-major fold: xT[p, k, t] = x[t, 128k+p])
            xT = xpool.tile([128, KC, TC], BF16, tag="xT")
            nc.sync.dma_start(
                out=xT[:, :, :],
                in_=x_bf[b, tok0 : tok0 + TC, :],
                transpose=True,
            )

            # qT[d_attn, tok] = Wq.T @ xT  (two psum banks per eviction)
            qT = qpool.tile([128, MC, TC], BF16, tag="qT")
            for gg in range(2):
                pq = pqp.tile([128, 2, TC], F32, tag="pq")
                for g2 in range(2):
                    mcol = 2 * gg + g2
                    for k in range(KC):
                        nc.tensor.matmul(
                            pq[:, g2, :],
                            Wq_bf[:, k, 128 * mcol : 128 * (mcol + 1)],
                            xT[:, k, :],
                            start=(k == 0),
                            stop=(k == KC - 1),
                        )
                nc.scalar.copy(qT[:, 2 * gg : 2 * gg + 2, :], pq[:, :, :])

            # scores + exp per head pair (row-tiled concurrent matmuls);
            # per-head colsums accumulate on PE into sm
            E = epool.tile([128, H, TC], BF16, tag="E")
            sm = pqp.tile([8, TC], F32, tag="pq")
            for hp in range(HPAIRS):
                ss = pmain.tile([128, 2, TC], F32, tag="mm")
                nc.tensor.matmul(
                    ss[:M, 0, :],
                    kT[0:64, hp, :],
                    qT[0:64, hp, :],
                    start=True,
                    stop=True,
                )
                nc.tensor.matmul(
                    ss[:M, 1, :],
                    kT[64:128, hp, :],
                    qT[64:128, hp, :],
                    start=True,
                    stop=True,
                )
                nc.scalar.activation(
                    E[:M, 2 * hp : 2 * hp + 2, :],
                    ss[:M, :, :],
                    func=EXP,
                    scale=DH**-0.5,
                )
                for j in range(2):
                    h = 2 * hp + j
                    nc.tensor.matmul(
                        sm,
                        colpick[:M, 8 - h : 16 - h],
                        E[:M, h, :],
                        start=(h == 0),
                        stop=(h == H - 1),
                    )

            # 1/sums -> bf16 -> PE broadcast across partitions (no DRAM bounce)
            r8f = rpool.tile([8, TC], F32, tag="r8f")
            nc.vector.reciprocal_approx_fast(out=r8f[:8, :], in_=sm[:8, :])
            r8 = rpool.tile([8, TC], BF16, tag="r8")
            nc.vector.tensor_scalar_mul(r8[:8, :], r8f[:8, :], 1.0)
            rss = []
            for ri in range(2):
                pb = psbp.tile([128, 2, TC], F32, tag="pb")
                for j in range(2):
                    nc.tensor.matmul(
                        pb[:, j, :], sel[:8, 2 * ri + j, :], r8[:8, :],
                        start=True, stop=True,
                    )
                rs = rpool.tile([128, 2, TC], BF16, tag="rs")
                nc.scalar.copy(rs, pb)
                rss.append(rs)

            # attn @ v (col-tiled pairs), normalized straight out of PSUM
            aT = apool.tile([128, MC, TC], BF16, tag="aT")
            for ai in range(2):
                po = pmain.tile([128, 2, TC], F32, tag="mm")
                for j in range(2):
                    hp = 2 * ai + j
                    nc.tensor.matmul(
                        po[0:64, j, :],
                        v_bf[:M, 128 * hp : 128 * hp + 64],
                        E[:M, 2 * hp, :],
                        start=True,
                        stop=True,
                    )
                    nc.tensor.matmul(
                        po[64:128, j, :],
                        v_bf[:M, 128 * hp + 64 : 128 * (hp + 1)],
                        E[:M, 2 * hp + 1, :],
                        start=True,
                        stop=True,
                    )
                for j in range(2):
                    hp = 2 * ai + j
                    nc.vector.tensor_mul(aT[:, hp, :], po[:, j, :], rss[ai][:, j, :])

            # out = aT.T @ Wo + bo  (aT chunks stationary -> token-major psum)
            bo2 = bass.AP(
                tensor=bo_bc.tensor,
                offset=bo_bc.offset,
                ap=[bo_bc.ap[0], [0, 2], *bo_bc.ap[1:]],
            )
            for uu in range(2):
                pu = pmain.tile([128, 2, FEAT], F32, tag="mm")
                for s2 in range(2):
                    s = 2 * uu + s2
                    for m in range(MC):
                        nc.tensor.matmul(
                            pu[:, s2, :],
                            aT[:, m, 128 * s : 128 * (s + 1)],
                            Wo_bf[:, m, :],
                            start=(m == 0),
                            stop=(m == MC - 1),
                        )
                osb = opool.tile([128, 2, FEAT], F32, tag="osb")
                nc.vector.tensor_add(osb, pu, bo2)
                r0 = tok0 + 256 * uu
                oslice = out[b, r0 : r0 + 256, :]
                out_ap = bass.AP(
                    tensor=oslice.tensor,
                    offset=oslice.offset,
                    ap=[[FEAT, 128], [128 * FEAT, 2], [1, FEAT]],
                )
                nc.sync.dma_start(out=out_ap, in_=osb)


def build():
    nc = bacc.Bacc(
        "TRN2", target_bir_lowering=False, debug=False, num_devices=N_CORES
    )
    x = nc.dram_tensor("x", [BP, N, FEAT], F32, kind="ExternalInput").ap()
    cond = nc.dram_tensor("cond", [BP, M, CD], F32, kind="ExternalInput").ap()
    Wq = nc.dram_tensor("Wq", [FEAT, DA], F32, kind="ExternalInput").ap()
    Wk = nc.dram_tensor("Wk", [CD, DA], F32, kind="ExternalInput").ap()
    Wv = nc.dram_tensor("Wv", [CD, DA], F32, kind="ExternalInput").ap()
    Wo = nc.dram_tensor("Wo", [DA, FEAT], F32, kind="ExternalInput").ap()
    bo = nc.dram_tensor("bo", [FEAT], F32, kind="ExternalInput").ap()
    out = nc.dram_tensor("out", [BP, N, FEAT], F32, kind="ExternalOutput").ap()
    x_bf = nc.dram_tensor("x_bf16_stage", [BP, N, FEAT], BF16).ap()
    with tile.TileContext(nc) as tc:
        _body(tc, x, x_bf, cond, Wq, Wk, Wv, Wo, bo, out)
    nc.compile()
    return nc


_NC = None


def kernel(x, cond, Wq, Wk, Wv, Wo, bo, _trace=False):
    global _NC
    if _NC is None:
        _NC = build()
    shared = {
        "Wq": np.asarray(Wq, np.float32),
        "Wk": np.asarray(Wk, np.float32),
        "Wv": np.asarray(Wv, np.float32),
        "Wo": np.asarray(Wo, np.float32),
        "bo": np.asarray(bo, np.float32),
    }
    in_maps = [
        {
            "x": np.ascontiguousarray(x[BP * i : BP * (i + 1)], dtype=np.float32),
            "cond": np.ascontiguousarray(cond[BP * i : BP * (i + 1)], dtype=np.float32),
            **shared,
        }
        for i in range(N_CORES)
    ]
    res = run_bass_kernel_spmd(_NC, in_maps, list(range(N_CORES)), trace=_trace)
    out = np.concatenate([r["out"] for r in res.results], axis=0)
    if _trace:
        kernel.last_exec_time_ns = res.exec_time_ns
        kernel.last_results = res
    return out
